# revision 8
# baseline (speedup 1.0000x reference)
"""Matrix NMS (SOLOv2 gaussian decay) on 8 TRN2 NeuronCores.

Strategy: shard the pixel (h*w=40960) contraction dim across the 8 cores.
The host pre-casts the binary masks to fp8 (exact for 0/1), so each core
DMAs only 5.24MB (vs 21MB f32) and the PE consumes it directly with
DoubleRow fp8 matmuls.  Each core computes the partial Gram upper block
rows; block-row a drains as int16 straight into AllToAll shard a (one
contiguous [128, N-128a] write, ~2KB descriptors).  After the AllToAll
(whose output AP interleaves peers into [row, peer, col] so the receiver
pulls all 8 partials of its block-row in ONE 16KB-per-partition DMA),
each core tree-sums its block-row and runs a row-oriented epilogue:
compensate_iou and the final min are partition-dim reductions done with
log2 halving maxes on the DVE, combined across cores by two 4KB
AllReduce(max) collectives.  Mask areas ride along as an extra row per
shard (diag of the Gram = area for 0/1 masks).  All core-dependent
constants (row masks, diagonal selector) are host-prepared inputs, so
the SPMD program is identical on every core; core 0's full [1,1024]
output is returned.
"""

import sys

import numpy as np

for _p in ("/opt/trn_rl_repo",):
    if _p not in sys.path:
        sys.path.insert(0, _p)

from concourse import bacc, bass, mybir, tile
from concourse import bass_utils

N = 1024           # candidates
HWPIX = 160 * 256  # 40960 pixels
W = 8              # cores
KC = HWPIX // W    # 5120 pixel-slice per core
KT = KC // 128     # 40 k-tiles of 128
GRP = 4            # k-tiles per resident SBUF group
RB = 128           # block-row height
SR = RB + 1        # shard rows: 128 gram rows + 1 area row
NP = KT // 2       # 20 k-tile pairs (DoubleRow)
SIGMA = 2.0

F32 = mybir.dt.float32
FP8 = mybir.dt.float8e4  # e4m3: exact for 0/1 mask values
I16 = mybir.dt.int16


def build_nc(variant="full"):
    # variant: "full" = real kernel; "nocc" = collectives replaced by local
    # DMA copies (wrong math, identical local compute/DMA — timing/sim only)
    nc = bacc.Bacc(
        "TRN2", target_bir_lowering=False, debug=False,
        num_devices=W if variant == "full" else 1,
    )

    xT = nc.dram_tensor("xT", [128, KT * N], FP8, kind="ExternalInput")
    maskR_h = nc.dram_tensor("maskR", [RB, N], F32, kind="ExternalInput")
    diagsel_h = nc.dram_tensor("diagsel", [RB, N], F32, kind="ExternalInput")
    scores_h = nc.dram_tensor("scores", [1, N], F32, kind="ExternalInput")
    ident_h = nc.dram_tensor("ident", [128, 128], F32, kind="ExternalInput")
    ones_h = nc.dram_tensor("ones_r", [1, 128], F32, kind="ExternalInput")
    out_h = nc.dram_tensor("out", [1, N], F32, kind="ExternalOutput")

    RG = [list(range(W))]

    with tile.TileContext(nc) as tc:
        with (
            tc.tile_pool(name="dram", bufs=1, space="DRAM") as dramp,
            tc.tile_pool(name="xp", bufs=1) as xp,
            tc.tile_pool(name="pg", bufs=4, space="PSUM") as pgp,
            tc.tile_pool(name="gb", bufs=4) as gbp,
            tc.tile_pool(name="a2al", bufs=1) as alp,
            tc.tile_pool(name="sc", bufs=1) as scp,
            tc.tile_pool(name="epi", bufs=1) as ep,
        ):
            cc_in = dramp.tile([W * SR, N], I16, tag="cc_in")
            a2a = dramp.tile([W * SR, N], I16, tag="a2a")
            ar1_in = dramp.tile([1, N], F32, tag="ar1_in")
            ar1_out = dramp.tile([1, N], F32, tag="ar1_out")
            ar2_in = dramp.tile([1, N], F32, tag="ar2_in")
            ar2_out = dramp.tile([1, N], F32, tag="ar2_out")

            # small constants + epilogue inputs (gpsimd queue, overlap phase 1)
            ident = scp.tile([128, 128], F32, tag="ident")
            nc.gpsimd.dma_start(ident[:], ident_h[:])
            ones_r = scp.tile([1, 128], F32, tag="ones_r")
            nc.gpsimd.dma_start(ones_r[:], ones_h[:])
            maskR = scp.tile([128, N], F32, tag="maskR")
            nc.gpsimd.dma_start(maskR[:], maskR_h[:])
            diagsel = scp.tile([128, N], F32, tag="diagsel")
            nc.gpsimd.dma_start(diagsel[:], diagsel_h[:])
            scores = scp.tile([1, N], F32, tag="scores")
            nc.gpsimd.dma_start(scores[:], scores_h[:])
            s_all = scp.tile([128, W], F32, tag="s_all")

            # ---- phase 1: fp8 x slice straight into SBUF (10 group loads)
            xg = [xp.tile([128, GRP, N], FP8, tag=f"x{g}", name=f"xg{g}")
                  for g in range(KT // GRP)]
            for g in range(KT // GRP):
                nc.sync.dma_start(xg[g][:], xT[:, g * GRP * N : (g + 1) * GRP * N])

            def xpair(q, c0, c1):
                t = 2 * q
                g, j = t // GRP, t % GRP
                return xg[g][:, j : j + 2, c0:c1]

            def gram_pair(pg, a, q):
                wdt = N - a * 128
                lhsT = xpair(q, a * 128, (a + 1) * 128)
                for off in range(0, wdt, 512):
                    cw = min(512, wdt - off)
                    nc.tensor.matmul(
                        pg[:, off : off + cw],
                        lhsT,
                        xpair(q, a * 128 + off, a * 128 + off + cw),
                        start=(q == 0),
                        stop=(q == NP - 1),
                        perf_mode=mybir.MatmulPerfMode.DoubleRow,
                    )

            def drain(a, pg):
                """PSUM block-row a -> int16 rows of AllToAll shard a.

                Shard a IS block-row a, so the write is one [128, N-128a]
                slab with per-partition-contiguous DRAM runs (up to 2KB).
                The left columns (<128a) of the shard stay garbage; the
                receiver's row mask zeroes them before any reduction.
                """
                wdt = N - a * 128
                gb16 = gbp.tile([128, wdt], I16, tag="gb16")
                nc.vector.tensor_copy(gb16[:], pg[:, :wdt])
                nc.sync.dma_start(cc_in[a * SR : a * SR + 128, a * 128 : N], gb16[:])
                # diag block -> partial areas (X is 0/1 so diag(Gram) = area)
                dmul = gbp.tile([128, 128], F32, tag="dmul")
                nc.vector.tensor_mul(dmul[:], pg[:, 0:128], ident[:])
                nc.vector.tensor_reduce(
                    s_all[:, a : a + 1], dmul[:], axis=mybir.AxisListType.X,
                    op=mybir.AluOpType.add,
                )

            # ---- phase 2: Gram upper block-rows in two PSUM waves
            wave_a = [pgp.tile([128, N - a * 128], F32, tag="pg", name=f"pgA{a}")
                      for a in range(4)]
            for q in range(NP):
                for a in range(4):
                    gram_pair(wave_a[a], a, q)
            for a in range(4):
                drain(a, wave_a[a])
            for a in range(4, W):
                pg = pgp.tile([128, N - a * 128], F32, tag="pg")
                for q in range(NP):
                    gram_pair(pg, a, q)
                drain(a, pg)

            # partial areas: transpose (128,W) -> (W,128) on the PE so the
            # area rows leave SBUF as contiguous 256B rows, not 4B gathers
            s_ps = pgp.tile([W, 128], F32, tag="pg", name="s_ps")
            nc.tensor.transpose(s_ps[:], s_all[:], ident[:])
            s_rt = scp.tile([W, 128], I16, tag="s_rt")
            nc.vector.tensor_copy(s_rt[:], s_ps[:])
            for r in range(W):
                nc.scalar.dma_start(cc_in[r * SR + RB : r * SR + RB + 1, :], s_rt[:])

            # ---- AllToAll: shard s of the output = core s's partial of MY
            # block-row (+ its partial area row)
            if variant == "full":
                nc.gpsimd.collective_compute(
                    "AllToAll",
                    mybir.AluOpType.bypass,
                    replica_groups=RG,
                    ins=[cc_in[:].opt()],
                    outs=[a2a[:].opt()],
                )
            else:
                for s in range(W):
                    nc.sync.dma_start(
                        a2a[s * SR : (s + 1) * SR, :], cc_in[s * SR : (s + 1) * SR, :]
                    )

            # ---- local tree-sum of the 8 partials of this core's block-row
            # (4 pair-loads on alternating queues, 2KB descriptors)
            lv = []
            for h in range(4):
                lt = alp.tile([RB, 2, N], I16, tag=f"ld{h}", name=f"ld{h}")
                src = a2a[2 * h * SR : (2 * h + 2) * SR, :].rearrange(
                    "(s p) n -> p s n", p=SR
                )[0:RB, :, :]
                eng = nc.sync if h % 2 == 0 else nc.scalar
                eng.dma_start(lt[:], src)
                lv.append(lt)
            m01 = ep.tile([RB, 2, N], I16, tag="m01")
            nc.vector.tensor_add(m01[:], lv[0][:], lv[1][:])
            m23 = ep.tile([RB, 2, N], I16, tag="m23")
            nc.vector.tensor_add(m23[:], lv[2][:], lv[3][:])
            p2 = ep.tile([RB, 2, N], I16, tag="p2")
            nc.vector.tensor_add(p2[:], m01[:], m23[:])
            summ = ep.tile([128, N], F32, tag="summ")
            nc.vector.tensor_add(summ[:], p2[:, 0, :], p2[:, 1, :])
            # area rows: partition-dim sum on gpsimd (8 -> 1)
            ar = ep.tile([W, N], I16, tag="ar")
            nc.gpsimd.dma_start(
                ar[:],
                a2a[:].rearrange("(s p) n -> s p n", p=SR)[:, RB, :],
            )
            srow = ep.tile([1, N], F32, tag="srow")
            nc.gpsimd.tensor_reduce(
                srow[:], ar[:], axis=mybir.AxisListType.C, op=mybir.AluOpType.add
            )

            # ---- row-oriented epilogue on the block-row
            # s_col[p] = area of own row p (diagonal of the block-row)
            tmp = ep.tile([128, N], F32, tag="tmp")
            nc.vector.tensor_mul(tmp[:], summ[:], diagsel[:])
            s_col = ep.tile([128, 1], F32, tag="s_col")
            nc.vector.tensor_reduce(
                s_col[:], tmp[:], axis=mybir.AxisListType.X, op=mybir.AluOpType.add
            )
            # broadcast srow across partitions via k=1 outer matmul
            sj = pgp.tile([128, N], F32, tag="pg", name="sj")
            for off in range(0, N, 512):
                nc.tensor.matmul(
                    sj[:, off : off + 512], ones_r[:], srow[:, off : off + 512],
                    start=True, stop=True,
                )
            # union = s_i + s_j - inter, clamped >= 1
            un = ep.tile([128, N], F32, tag="un")
            nc.vector.tensor_tensor(un[:], sj[:], summ[:], op=mybir.AluOpType.subtract)
            nc.vector.tensor_scalar(
                un[:], un[:], s_col[:], 1.0,
                op0=mybir.AluOpType.add, op1=mybir.AluOpType.max,
            )
            rec = ep.tile([128, N], F32, tag="rec")
            nc.vector.reciprocal_approx_fast(rec[:], un[:])
            # d[p, j] = masked IoU of own row p vs candidate j
            d = ep.tile([128, N], F32, tag="d")
            nc.vector.tensor_mul(d[:], summ[:], rec[:])
            nc.vector.tensor_mul(d[:], d[:], maskR[:])

            # partial compensate_iou (column max over own rows) -> AllReduce
            pc1 = ep.tile([1, N], F32, tag="pc1")
            nc.gpsimd.tensor_reduce(
                pc1[:], d[:], axis=mybir.AxisListType.C, op=mybir.AluOpType.max
            )
            nc.sync.dma_start(ar1_in[:], pc1[:])
            if variant == "full":
                nc.gpsimd.collective_compute(
                    "AllReduce",
                    mybir.AluOpType.max,
                    replica_groups=RG,
                    ins=[ar1_in[:].opt()],
                    outs=[ar1_out[:].opt()],
                )
            else:
                nc.sync.dma_start(ar1_out[:], ar1_in[:])
            c_red = ep.tile([1, N], F32, tag="c_red")
            nc.sync.dma_start(c_red[:], ar1_out[:])
            # c_own[p] = compensate at own row p (diagonal extract)
            cb = pgp.tile([128, N], F32, tag="pg", name="cb")
            for off in range(0, N, 512):
                nc.tensor.matmul(
                    cb[:, off : off + 512], ones_r[:], c_red[:, off : off + 512],
                    start=True, stop=True,
                )
            ct = ep.tile([128, N], F32, tag="ct")
            nc.vector.tensor_mul(ct[:], cb[:], diagsel[:])
            c_own = ep.tile([128, 1], F32, tag="c_own")
            nc.vector.tensor_reduce(
                c_own[:], ct[:], axis=mybir.AxisListType.X, op=mybir.AluOpType.add
            )
            c2 = ep.tile([128, 1], F32, tag="c2")
            nc.vector.tensor_mul(c2[:], c_own[:], c_own[:])
            # f[p, j] = d^2 - c_own^2 ; M[j] = global max -> coeff = exp(-s M)
            f = ep.tile([128, N], F32, tag="f")
            nc.vector.tensor_mul(f[:], d[:], d[:])
            nc.vector.tensor_scalar(
                f[:], f[:], c2[:], None, op0=mybir.AluOpType.subtract
            )
            pm1 = ep.tile([1, N], F32, tag="pm1")
            nc.gpsimd.tensor_reduce(
                pm1[:], f[:], axis=mybir.AxisListType.C, op=mybir.AluOpType.max
            )
            nc.sync.dma_start(ar2_in[:], pm1[:])
            if variant == "full":
                nc.gpsimd.collective_compute(
                    "AllReduce",
                    mybir.AluOpType.max,
                    replica_groups=RG,
                    ins=[ar2_in[:].opt()],
                    outs=[ar2_out[:].opt()],
                )
            else:
                nc.sync.dma_start(ar2_out[:], ar2_in[:])
            m_red = ep.tile([1, N], F32, tag="m_red")
            nc.sync.dma_start(m_red[:], ar2_out[:])
            coeff = ep.tile([1, N], F32, tag="coeff")
            nc.scalar.activation(
                coeff[:], m_red[:], mybir.ActivationFunctionType.Exp, scale=-SIGMA
            )
            outsb = ep.tile([1, N], F32, tag="outsb")
            nc.vector.tensor_mul(outsb[:], coeff[:], scores[:])
            nc.scalar.dma_start(out_h[:], outsb[:])

    nc.compile()
    return nc


_NC_CACHE = {}


def _get_nc(variant="full"):
    if variant not in _NC_CACHE:
        _NC_CACHE[variant] = build_nc(variant)
    return _NC_CACHE[variant]


def make_in_maps(seg_masks, cate_labels, cate_scores):
    import ml_dtypes

    flat = np.asarray(seg_masks, dtype=np.float32).reshape(N, -1)
    labels = np.asarray(cate_labels)
    scores = np.ascontiguousarray(
        np.asarray(cate_scores, dtype=np.float32).reshape(1, N)
    )
    xTfull = np.ascontiguousarray(flat.T)  # (40960, 1024)
    gidx = np.arange(N)
    ident = np.eye(128, dtype=np.float32)
    ones_r = np.ones((1, 128), dtype=np.float32)
    in_maps = []
    for c in range(W):
        rows = slice(c * RB, (c + 1) * RB)
        gr = gidx[rows]
        maskR = (
            (gidx[None, :] > gr[:, None]) & (labels[None, :] == labels[rows][:, None])
        ).astype(np.float32)
        diagsel = np.zeros((RB, N), dtype=np.float32)
        diagsel[np.arange(RB), gr] = 1.0
        in_maps.append(
            {
                # partition-major: row p holds k-rows {p, 128+p, ...} of this
                # core's slice; host casts to fp8 (exact for 0/1 masks)
                "xT": np.ascontiguousarray(
                    xTfull[c * KC : (c + 1) * KC]
                    .reshape(KT, 128, N)
                    .transpose(1, 0, 2)
                ).reshape(128, KT * N).astype(ml_dtypes.float8_e4m3fn),
                "maskR": maskR,
                "diagsel": diagsel,
                "scores": scores,
                "ident": ident,
                "ones_r": ones_r,
            }
        )
    return in_maps


def run_device(in_maps, trace=False):
    nc = _get_nc()
    res = bass_utils.run_bass_kernel_spmd(
        nc, in_maps, core_ids=list(range(W)), trace=trace
    )
    return res


def kernel(seg_masks, cate_labels, cate_scores):
    in_maps = make_in_maps(seg_masks, cate_labels, cate_scores)
    res = run_device(in_maps)
    # every core computes the full decayed-score row; take core 0's
    return np.asarray(res.results[0]["out"]).reshape(N).astype(np.float32)


# revision 14
# speedup vs baseline: 2.3274x; 2.3274x over previous
"""Matrix NMS (SOLOv2 gaussian decay) on 8 TRN2 NeuronCores.

Strategy: shard the pixel (h*w=40960) contraction dim across the 8 cores.
The host pre-casts the binary masks to fp8 (exact for 0/1), so each core
DMAs only 5.24MB (vs 21MB f32) and the PE consumes it directly with
DoubleRow fp8 matmuls.  Each core computes the partial Gram upper block
rows; block-row a drains as int16 straight into AllToAll shard a (one
contiguous [128, N-128a] write, ~2KB descriptors).  After the AllToAll
(whose output AP interleaves peers into [row, peer, col] so the receiver
pulls all 8 partials of its block-row in ONE 16KB-per-partition DMA),
each core tree-sums its block-row and runs a row-oriented epilogue:
compensate_iou and the final min are partition-dim reductions done with
log2 halving maxes on the DVE, combined across cores by two 4KB
AllReduce(max) collectives.  Mask areas ride along as an extra row per
shard (diag of the Gram = area for 0/1 masks).  All core-dependent
constants (row masks, diagonal selector) are host-prepared inputs, so
the SPMD program is identical on every core; core 0's full [1,1024]
output is returned.
"""

import sys

import numpy as np

for _p in ("/opt/trn_rl_repo",):
    if _p not in sys.path:
        sys.path.insert(0, _p)

from concourse import bacc, bass, mybir, tile
from concourse import bass_utils

N = 1024           # candidates
HWPIX = 160 * 256  # 40960 pixels
W = 8              # cores
KC = HWPIX // W    # 5120 pixel-slice per core
KT = KC // 128     # 40 k-tiles of 128
GRP = 4            # k-tiles per resident SBUF group
RB = 128           # block-row height
SR = RB + 1        # shard rows: 128 gram rows + 1 area row
NP = KT // 2       # 20 k-tile pairs (DoubleRow)
SIGMA = 2.0

F32 = mybir.dt.float32
FP8 = mybir.dt.float8e4  # e4m3: exact for 0/1 mask values
I16 = mybir.dt.int16


def build_nc(variant="full"):
    # variant: "full" = real kernel; "nocc" = collectives replaced by local
    # DMA copies (wrong math, identical local compute/DMA — timing/sim only)
    nc = bacc.Bacc(
        "TRN2", target_bir_lowering=False, debug=False,
        num_devices=W if variant == "full" else 1,
    )

    xT = nc.dram_tensor("xT", [128, KT * N], FP8, kind="ExternalInput")
    maskR_h = nc.dram_tensor("maskR", [RB, N], F32, kind="ExternalInput")
    diagsel_h = nc.dram_tensor("diagsel", [RB, N], F32, kind="ExternalInput")
    scores_h = nc.dram_tensor("scores", [1, N], F32, kind="ExternalInput")
    ident_h = nc.dram_tensor("ident", [128, 128], F32, kind="ExternalInput")
    ones_h = nc.dram_tensor("ones_r", [1, 128], F32, kind="ExternalInput")
    out_h = nc.dram_tensor("out", [1, N], F32, kind="ExternalOutput")

    RG = [list(range(W))]

    with tile.TileContext(nc) as tc:
        with (
            tc.tile_pool(name="dram", bufs=1, space="DRAM") as dramp,
            tc.tile_pool(name="xp", bufs=1) as xp,
            tc.tile_pool(name="pg", bufs=4, space="PSUM") as pgp,
            tc.tile_pool(name="gb", bufs=4) as gbp,
            tc.tile_pool(name="a2al", bufs=1) as alp,
            tc.tile_pool(name="sc", bufs=1) as scp,
            tc.tile_pool(name="epi", bufs=1) as ep,
        ):
            cc_in = dramp.tile([W * SR, N], I16, tag="cc_in")
            a2a = dramp.tile([W * SR, N], I16, tag="a2a")
            ar1_in = dramp.tile([1, N], F32, tag="ar1_in")
            ar1_out = dramp.tile([1, N], F32, tag="ar1_out")
            ar2_in = dramp.tile([1, N], F32, tag="ar2_in")
            ar2_out = dramp.tile([1, N], F32, tag="ar2_out")

            # small constants + epilogue inputs (gpsimd queue, overlap phase 1)
            ident = scp.tile([128, 128], F32, tag="ident")
            nc.gpsimd.dma_start(ident[:], ident_h[:])
            ones_r = scp.tile([1, 128], F32, tag="ones_r")
            nc.gpsimd.dma_start(ones_r[:], ones_h[:])
            maskR = scp.tile([128, N], F32, tag="maskR")
            nc.scalar.dma_start(maskR[:], maskR_h[:])
            diagsel = scp.tile([128, N], F32, tag="diagsel")
            nc.scalar.dma_start(diagsel[:], diagsel_h[:])
            scores = scp.tile([1, N], F32, tag="scores")
            nc.scalar.dma_start(scores[:], scores_h[:])
            s_all = scp.tile([128, W], F32, tag="s_all")

            # ---- phase 1: fp8 x slice straight into SBUF (10 group loads)
            xg = [xp.tile([128, GRP, N], FP8, tag=f"x{g}", name=f"xg{g}")
                  for g in range(KT // GRP)]
            for g in range(KT // GRP):
                nc.sync.dma_start(xg[g][:], xT[:, g * GRP * N : (g + 1) * GRP * N])

            def xpair(q, c0, c1):
                t = 2 * q
                g, j = t // GRP, t % GRP
                return xg[g][:, j : j + 2, c0:c1]

            def gram_pair(pg, a, q):
                wdt = N - a * 128
                lhsT = xpair(q, a * 128, (a + 1) * 128)
                for off in range(0, wdt, 512):
                    cw = min(512, wdt - off)
                    nc.tensor.matmul(
                        pg[:, off : off + cw],
                        lhsT,
                        xpair(q, a * 128 + off, a * 128 + off + cw),
                        start=(q == 0),
                        stop=(q == NP - 1),
                        perf_mode=mybir.MatmulPerfMode.DoubleRow,
                    )

            def drain(a, pg):
                """PSUM block-row a -> int16 rows of AllToAll shard a.

                Shard a IS block-row a, so the write is one [128, N-128a]
                slab with per-partition-contiguous DRAM runs (up to 2KB).
                The left columns (<128a) of the shard stay garbage; the
                receiver's row mask zeroes them before any reduction.
                """
                wdt = N - a * 128
                gb16 = gbp.tile([128, wdt], I16, tag="gb16")
                nc.vector.tensor_copy(gb16[:], pg[:, :wdt])
                eng = (nc.sync, nc.scalar, nc.gpsimd)[a % 3]
                eng.dma_start(cc_in[a * SR : a * SR + 128, a * 128 : N], gb16[:])
                # diag block -> partial areas (X is 0/1 so diag(Gram) = area)
                dmul = gbp.tile([128, 128], F32, tag="dmul")
                nc.vector.tensor_mul(dmul[:], pg[:, 0:128], ident[:])
                nc.vector.tensor_reduce(
                    s_all[:, a : a + 1], dmul[:], axis=mybir.AxisListType.X,
                    op=mybir.AluOpType.add,
                )

            # ---- phase 2: Gram upper block-rows in two PSUM waves
            wave_a = [pgp.tile([128, N - a * 128], F32, tag="pg", name=f"pgA{a}")
                      for a in range(4)]
            for q in range(NP):
                for a in range(4):
                    gram_pair(wave_a[a], a, q)
            for a in range(4):
                drain(a, wave_a[a])
            for a in range(4, W):
                pg = pgp.tile([128, N - a * 128], F32, tag="pg")
                for q in range(NP):
                    gram_pair(pg, a, q)
                drain(a, pg)

            # partial areas: transpose (128,W) -> (W,128) on the PE so the
            # area rows leave SBUF as contiguous 256B rows, not 4B gathers
            s_ps = pgp.tile([W, 128], F32, tag="pg", name="s_ps")
            nc.tensor.transpose(s_ps[:], s_all[:], ident[:])
            s_rt = scp.tile([W, 128], I16, tag="s_rt")
            nc.vector.tensor_copy(s_rt[:], s_ps[:])
            for r in range(W):
                nc.scalar.dma_start(cc_in[r * SR + RB : r * SR + RB + 1, :], s_rt[:])

            # ---- AllToAll: shard s of the output = core s's partial of MY
            # block-row (+ its partial area row)
            if variant == "full":
                nc.gpsimd.collective_compute(
                    "AllToAll",
                    mybir.AluOpType.bypass,
                    replica_groups=RG,
                    ins=[cc_in[:].opt()],
                    outs=[a2a[:].opt()],
                )
            else:
                for s in range(W):
                    nc.sync.dma_start(
                        a2a[s * SR : (s + 1) * SR, :], cc_in[s * SR : (s + 1) * SR, :]
                    )

            # ---- local tree-sum of the 8 partials of this core's block-row
            # (4 pair-loads on alternating queues, 2KB descriptors)
            lv = []
            for h in range(4):
                lt = alp.tile([RB, 2, N], I16, tag=f"ld{h}", name=f"ld{h}")
                src = a2a[2 * h * SR : (2 * h + 2) * SR, :].rearrange(
                    "(s p) n -> p s n", p=SR
                )[0:RB, :, :]
                eng = nc.sync if h % 2 == 0 else nc.scalar
                eng.dma_start(lt[:], src)
                lv.append(lt)
            m01 = ep.tile([RB, 2, N], I16, tag="m01")
            nc.vector.tensor_add(m01[:], lv[0][:], lv[1][:])
            m23 = ep.tile([RB, 2, N], I16, tag="m23")
            nc.vector.tensor_add(m23[:], lv[2][:], lv[3][:])
            p2 = ep.tile([RB, 2, N], I16, tag="p2")
            nc.vector.tensor_add(p2[:], m01[:], m23[:])
            summ = ep.tile([128, N], F32, tag="summ")
            nc.vector.tensor_add(summ[:], p2[:, 0, :], p2[:, 1, :])
            # area rows: sum the 8 partials with a k=8 ones-matmul
            ar = ep.tile([W, N], I16, tag="ar")
            nc.gpsimd.dma_start(
                ar[:],
                a2a[:].rearrange("(s p) n -> s p n", p=SR)[:, RB, :],
            )
            arf = ep.tile([W, N], F32, tag="arf")
            nc.vector.tensor_copy(arf[:], ar[:])
            ones8 = ep.tile([W, 1], F32, tag="ones8")
            nc.vector.memset(ones8[:], 1.0)
            arp = pgp.tile([1, N], F32, tag="pg", name="arp")
            for off in range(0, N, 512):
                nc.tensor.matmul(
                    arp[:, off : off + 512], ones8[:], arf[:, off : off + 512],
                    start=True, stop=True,
                )
            srow = ep.tile([1, N], F32, tag="srow")
            nc.vector.tensor_copy(srow[:], arp[:])

            # ---- row-oriented epilogue on the block-row
            # s_col[p] = area of own row p (diagonal of the block-row)
            tmp = ep.tile([128, N], F32, tag="tmp")
            nc.vector.tensor_mul(tmp[:], summ[:], diagsel[:])
            s_col = ep.tile([128, 1], F32, tag="s_col")
            nc.vector.tensor_reduce(
                s_col[:], tmp[:], axis=mybir.AxisListType.X, op=mybir.AluOpType.add
            )
            # broadcast srow across partitions via k=1 outer matmul
            sj = pgp.tile([128, N], F32, tag="pg", name="sj")
            for off in range(0, N, 512):
                nc.tensor.matmul(
                    sj[:, off : off + 512], ones_r[:], srow[:, off : off + 512],
                    start=True, stop=True,
                )
            # union = s_i + s_j - inter, clamped >= 1
            un = ep.tile([128, N], F32, tag="un")
            nc.vector.tensor_tensor(un[:], sj[:], summ[:], op=mybir.AluOpType.subtract)
            nc.vector.tensor_scalar(
                un[:], un[:], s_col[:], 1.0,
                op0=mybir.AluOpType.add, op1=mybir.AluOpType.max,
            )
            rec = ep.tile([128, N], F32, tag="rec")
            nc.vector.reciprocal_approx_fast(rec[:], un[:])
            # d[p, j] = masked IoU of own row p vs candidate j
            d = ep.tile([128, N], F32, tag="d")
            nc.vector.tensor_mul(d[:], summ[:], rec[:])
            nc.vector.tensor_mul(d[:], d[:], maskR[:])

            def col_max(src, name):
                """[128, N] -> [W, 128] column max (flattens to global col
                order) via PE chunk transposes + DVE free-dim reduce."""
                tp = pgp.tile([128, W, 128], F32, tag="pg", name=f"{name}_tp")
                for k in range(W):
                    nc.tensor.transpose(
                        tp[:, k, :], src[:, k * 128 : (k + 1) * 128], ident[:]
                    )
                mx8 = ep.tile([128, W], F32, tag=f"{name}_mx8")
                nc.vector.tensor_reduce(
                    mx8[:], tp[:], axis=mybir.AxisListType.X, op=mybir.AluOpType.max
                )
                m8ps = pgp.tile([W, 128], F32, tag="pg", name=f"{name}_m8ps")
                nc.tensor.transpose(m8ps[:], mx8[:], ident[:])
                m8s = ep.tile([W, 128], F32, tag=f"{name}_m8s")
                nc.vector.tensor_copy(m8s[:], m8ps[:])
                return m8s

            # partial compensate_iou (column max over own rows) -> AllReduce
            pc1 = col_max(d, "pc")
            nc.sync.dma_start(ar1_in[:], pc1[:])
            # d^2 on DVE now, so it overlaps the AllReduce below
            f = ep.tile([128, N], F32, tag="f")
            nc.vector.tensor_mul(f[:], d[:], d[:])
            if variant == "full":
                nc.gpsimd.collective_compute(
                    "AllReduce",
                    mybir.AluOpType.max,
                    replica_groups=RG,
                    ins=[ar1_in[:].opt()],
                    outs=[ar1_out[:].opt()],
                )
            else:
                nc.sync.dma_start(ar1_out[:], ar1_in[:])
            c_red = ep.tile([1, N], F32, tag="c_red")
            nc.sync.dma_start(c_red[:], ar1_out[:])
            # c_own[p] = compensate at own row p (diagonal extract)
            cb = pgp.tile([128, N], F32, tag="pg", name="cb")
            for off in range(0, N, 512):
                nc.tensor.matmul(
                    cb[:, off : off + 512], ones_r[:], c_red[:, off : off + 512],
                    start=True, stop=True,
                )
            ct = ep.tile([128, N], F32, tag="ct")
            nc.vector.tensor_mul(ct[:], cb[:], diagsel[:])
            c_own = ep.tile([128, 1], F32, tag="c_own")
            nc.vector.tensor_reduce(
                c_own[:], ct[:], axis=mybir.AxisListType.X, op=mybir.AluOpType.add
            )
            c2 = ep.tile([128, 1], F32, tag="c2")
            nc.vector.tensor_mul(c2[:], c_own[:], c_own[:])
            # f[p, j] = d^2 - c_own^2 ; M[j] = global max -> coeff = exp(-s M)
            nc.vector.tensor_scalar(
                f[:], f[:], c2[:], None, op0=mybir.AluOpType.subtract
            )
            pm1 = col_max(f, "pm")
            nc.sync.dma_start(ar2_in[:], pm1[:])
            if variant == "full":
                nc.gpsimd.collective_compute(
                    "AllReduce",
                    mybir.AluOpType.max,
                    replica_groups=RG,
                    ins=[ar2_in[:].opt()],
                    outs=[ar2_out[:].opt()],
                )
            else:
                nc.sync.dma_start(ar2_out[:], ar2_in[:])
            m_red = ep.tile([1, N], F32, tag="m_red")
            nc.sync.dma_start(m_red[:], ar2_out[:])
            coeff = ep.tile([1, N], F32, tag="coeff")
            nc.scalar.activation(
                coeff[:], m_red[:], mybir.ActivationFunctionType.Exp, scale=-SIGMA
            )
            outsb = ep.tile([1, N], F32, tag="outsb")
            nc.vector.tensor_mul(outsb[:], coeff[:], scores[:])
            nc.scalar.dma_start(out_h[:], outsb[:])

    nc.compile()
    return nc


_NC_CACHE = {}


def _get_nc(variant="full"):
    if variant not in _NC_CACHE:
        _NC_CACHE[variant] = build_nc(variant)
    return _NC_CACHE[variant]


def make_in_maps(seg_masks, cate_labels, cate_scores):
    import ml_dtypes

    flat = np.asarray(seg_masks, dtype=np.float32).reshape(N, -1)
    labels = np.asarray(cate_labels)
    scores = np.ascontiguousarray(
        np.asarray(cate_scores, dtype=np.float32).reshape(1, N)
    )
    xTfull = np.ascontiguousarray(flat.T)  # (40960, 1024)
    gidx = np.arange(N)
    ident = np.eye(128, dtype=np.float32)
    ones_r = np.ones((1, 128), dtype=np.float32)
    in_maps = []
    for c in range(W):
        rows = slice(c * RB, (c + 1) * RB)
        gr = gidx[rows]
        maskR = (
            (gidx[None, :] > gr[:, None]) & (labels[None, :] == labels[rows][:, None])
        ).astype(np.float32)
        diagsel = np.zeros((RB, N), dtype=np.float32)
        diagsel[np.arange(RB), gr] = 1.0
        in_maps.append(
            {
                # partition-major: row p holds k-rows {p, 128+p, ...} of this
                # core's slice; host casts to fp8 (exact for 0/1 masks)
                "xT": np.ascontiguousarray(
                    xTfull[c * KC : (c + 1) * KC]
                    .reshape(KT, 128, N)
                    .transpose(1, 0, 2)
                ).reshape(128, KT * N).astype(ml_dtypes.float8_e4m3fn),
                "maskR": maskR,
                "diagsel": diagsel,
                "scores": scores,
                "ident": ident,
                "ones_r": ones_r,
            }
        )
    return in_maps


def run_device(in_maps, trace=False):
    nc = _get_nc()
    res = bass_utils.run_bass_kernel_spmd(
        nc, in_maps, core_ids=list(range(W)), trace=trace
    )
    return res


def kernel(seg_masks, cate_labels, cate_scores):
    in_maps = make_in_maps(seg_masks, cate_labels, cate_scores)
    res = run_device(in_maps)
    # every core computes the full decayed-score row; take core 0's
    return np.asarray(res.results[0]["out"]).reshape(N).astype(np.float32)


# revision 17
# speedup vs baseline: 2.9207x; 1.2549x over previous
"""Matrix NMS (SOLOv2 gaussian decay) on 8 TRN2 NeuronCores.

Strategy: shard the pixel (h*w=40960) contraction dim across the 8 cores.
The host pre-casts the binary masks to fp8 (exact for 0/1), so each core
DMAs only 5.24MB (vs 21MB f32) and the PE consumes it directly with
DoubleRow fp8 matmuls.  Each core computes the partial Gram upper block
rows; block-row a drains as int16 straight into AllToAll shard a (one
contiguous [128, N-128a] write, ~2KB descriptors, copies alternating
DVE/ACT, queues alternating sync/scalar/gpsimd).  After the AllToAll each
core pulls the 8 partials of its own block-row (2KB-descriptor loads on 3
queues) and tree-sums them.  The epilogue is row-oriented: union comes
from a host-precomputed area table (areas are just mask sums — host
input), masked IoU d is formed with one approx-reciprocal pass, and the
two cross-candidate reductions (compensate_iou column max, final decay
max) are done as PE-chunk-transposes + DVE free-dim max, combined across
cores by two 4KB ReduceScatter(max) collectives.  Every core ends up
with the decayed scores for its own 128 candidates; the host concatenates
the 8 slices.
"""

import sys

import numpy as np

for _p in ("/opt/trn_rl_repo",):
    if _p not in sys.path:
        sys.path.insert(0, _p)

from concourse import bacc, bass, mybir, tile
from concourse import bass_utils

N = 1024           # candidates
HWPIX = 160 * 256  # 40960 pixels
W = 8              # cores
KC = HWPIX // W    # 5120 pixel-slice per core
KT = KC // 128     # 40 k-tiles of 128
GRP = 4            # k-tiles per resident SBUF group
RB = 128           # block-row height == shard rows
NP = KT // 2       # 20 k-tile pairs (DoubleRow)
SIGMA = 2.0

F32 = mybir.dt.float32
FP8 = mybir.dt.float8e4  # e4m3: exact for 0/1 mask values
I16 = mybir.dt.int16


def build_nc(variant="full"):
    # variant: "full" = real kernel; "nocc" = collectives replaced by local
    # DMA copies (wrong math, identical local compute/DMA — timing/sim only)
    nc = bacc.Bacc(
        "TRN2", target_bir_lowering=False, debug=False,
        num_devices=W if variant == "full" else 1,
    )

    xT = nc.dram_tensor("xT", [128, KT * N], FP8, kind="ExternalInput")
    maskR_h = nc.dram_tensor("maskR", [RB, N], F32, kind="ExternalInput")
    sjsc_h = nc.dram_tensor("sjsc", [RB, N], F32, kind="ExternalInput")
    scores_h = nc.dram_tensor("scores", [1, RB], F32, kind="ExternalInput")
    ident_h = nc.dram_tensor("ident", [128, 128], F32, kind="ExternalInput")
    out_h = nc.dram_tensor("out", [1, RB], F32, kind="ExternalOutput")

    RG = [list(range(W))]

    with tile.TileContext(nc) as tc:
        with (
            tc.tile_pool(name="dram", bufs=1, space="DRAM") as dramp,
            tc.tile_pool(name="xp", bufs=1) as xp,
            tc.tile_pool(name="pg", bufs=4, space="PSUM") as pgp,
            tc.tile_pool(name="gb", bufs=4) as gbp,
            tc.tile_pool(name="a2al", bufs=1) as alp,
            tc.tile_pool(name="sc", bufs=1) as scp,
            tc.tile_pool(name="epi", bufs=1) as ep,
        ):
            cc_in = dramp.tile([W * RB, N], I16, tag="cc_in")
            a2a = dramp.tile([W * RB, N], I16, tag="a2a")
            rs1_in = dramp.tile([1, N], F32, tag="rs1_in")
            rs1_out = dramp.tile([1, RB], F32, tag="rs1_out")
            rs2_in = dramp.tile([1, N], F32, tag="rs2_in")
            rs2_out = dramp.tile([1, RB], F32, tag="rs2_out")

            # constants + epilogue inputs (off the sync queue used by x loads)
            ident = scp.tile([128, 128], F32, tag="ident")
            nc.gpsimd.dma_start(ident[:], ident_h[:])
            scores = scp.tile([1, RB], F32, tag="scores")
            nc.gpsimd.dma_start(scores[:], scores_h[:])
            maskR = scp.tile([128, N], F32, tag="maskR")
            nc.scalar.dma_start(maskR[:], maskR_h[:])
            sjsc = scp.tile([128, N], F32, tag="sjsc")
            nc.scalar.dma_start(sjsc[:], sjsc_h[:])

            # ---- phase 1: fp8 x slice straight into SBUF (10 group loads)
            xg = [xp.tile([128, GRP, N], FP8, tag=f"x{g}", name=f"xg{g}")
                  for g in range(KT // GRP)]
            for g in range(KT // GRP):
                nc.sync.dma_start(xg[g][:], xT[:, g * GRP * N : (g + 1) * GRP * N])

            def xpair(q, c0, c1):
                t = 2 * q
                g, j = t // GRP, t % GRP
                return xg[g][:, j : j + 2, c0:c1]

            def gram_pair(pg, a, q):
                wdt = N - a * 128
                lhsT = xpair(q, a * 128, (a + 1) * 128)
                for off in range(0, wdt, 512):
                    cw = min(512, wdt - off)
                    nc.tensor.matmul(
                        pg[:, off : off + cw],
                        lhsT,
                        xpair(q, a * 128 + off, a * 128 + off + cw),
                        start=(q == 0),
                        stop=(q == NP - 1),
                        perf_mode=mybir.MatmulPerfMode.DoubleRow,
                    )

            def drain(a, pg):
                """PSUM block-row a -> int16 rows of AllToAll shard a.

                Shard a IS block-row a, so the write is one [128, N-128a]
                slab with per-partition-contiguous DRAM runs (up to 2KB).
                The left columns (<128a) of the shard stay garbage; the
                receiver's row mask zeroes them before any reduction.
                """
                wdt = N - a * 128
                gb16 = gbp.tile([128, wdt], I16, tag="gb16")
                if a % 2 == 0:
                    nc.vector.tensor_copy(gb16[:], pg[:, :wdt])
                else:
                    nc.scalar.activation(
                        gb16[:], pg[:, :wdt], mybir.ActivationFunctionType.Copy
                    )
                eng = (nc.sync, nc.scalar, nc.gpsimd)[a % 3]
                eng.dma_start(cc_in[a * RB : (a + 1) * RB, a * 128 : N], gb16[:])

            # ---- phase 2: Gram upper block-rows in two PSUM waves
            wave_a = [pgp.tile([128, N - a * 128], F32, tag="pg", name=f"pgA{a}")
                      for a in range(4)]
            for q in range(NP):
                for a in range(4):
                    gram_pair(wave_a[a], a, q)
            for a in range(4):
                drain(a, wave_a[a])
            for a in range(4, W):
                pg = pgp.tile([128, N - a * 128], F32, tag="pg")
                for q in range(NP):
                    gram_pair(pg, a, q)
                drain(a, pg)

            # ---- AllToAll: shard s of the output = core s's partial of MY
            # block-row
            if variant == "full":
                nc.gpsimd.collective_compute(
                    "AllToAll",
                    mybir.AluOpType.bypass,
                    replica_groups=RG,
                    ins=[cc_in[:].opt()],
                    outs=[a2a[:].opt()],
                )
            else:
                nc.sync.dma_start(a2a[:], cc_in[:])

            # ---- local tree-sum of the 8 partials of this core's block-row
            # (4 pair-loads on alternating queues, 2KB descriptors)
            lv = []
            for h in range(4):
                lt = alp.tile([RB, 2, N], I16, tag=f"ld{h}", name=f"ld{h}")
                src = a2a[2 * h * RB : (2 * h + 2) * RB, :].rearrange(
                    "(s p) n -> p s n", p=RB
                )
                eng = (nc.sync, nc.scalar, nc.gpsimd, nc.sync)[h]
                eng.dma_start(lt[:], src)
                lv.append(lt)
            m01 = ep.tile([RB, 2, N], I16, tag="m01")
            nc.vector.tensor_add(m01[:], lv[0][:], lv[1][:])
            m23 = ep.tile([RB, 2, N], I16, tag="m23")
            nc.vector.tensor_add(m23[:], lv[2][:], lv[3][:])
            p2 = ep.tile([RB, 2, N], I16, tag="p2")
            nc.vector.tensor_add(p2[:], m01[:], m23[:])
            summ = ep.tile([128, N], F32, tag="summ")
            nc.vector.tensor_add(summ[:], p2[:, 0, :], p2[:, 1, :])

            # ---- row-oriented epilogue on the block-row
            # union = s_i + s_j - inter (host-precomputed s_i+s_j), clamp >= 1
            un = ep.tile([128, N], F32, tag="un")
            nc.vector.tensor_tensor(un[:], sjsc[:], summ[:], op=mybir.AluOpType.subtract)
            nc.vector.tensor_scalar(
                un[:], un[:], 1.0, None, op0=mybir.AluOpType.max
            )
            rec = ep.tile([128, N], F32, tag="rec")
            nc.vector.reciprocal_approx_fast(rec[:], un[:])
            # d[p, j] = masked IoU of own row p vs candidate j
            d = ep.tile([128, N], F32, tag="d")
            nc.vector.tensor_mul(d[:], summ[:], rec[:])
            nc.vector.tensor_mul(d[:], d[:], maskR[:])

            def col_max(src, name):
                """[128, N] -> [W, 128] column max (flattens to global col
                order) via PE chunk transposes + DVE free-dim reduce."""
                tp = pgp.tile([128, W, 128], F32, tag="pg", name=f"{name}_tp")
                for k in range(W):
                    nc.tensor.transpose(
                        tp[:, k, :], src[:, k * 128 : (k + 1) * 128], ident[:]
                    )
                mx8 = ep.tile([128, W], F32, tag=f"{name}_mx8")
                nc.vector.tensor_reduce(
                    mx8[:], tp[:], axis=mybir.AxisListType.X, op=mybir.AluOpType.max
                )
                m8ps = pgp.tile([W, 128], F32, tag="pg", name=f"{name}_m8ps")
                nc.tensor.transpose(m8ps[:], mx8[:], ident[:])
                m8s = ep.tile([W, 128], F32, tag=f"{name}_m8s")
                nc.vector.tensor_copy(m8s[:], m8ps[:])
                return m8s

            # partial compensate_iou (column max over own rows) ->
            # ReduceScatter(max): each core receives compensate for ITS rows
            pc1 = col_max(d, "pc")
            nc.sync.dma_start(rs1_in[:], pc1[:])
            # d^2 on DVE now, so it overlaps the collective below
            f = ep.tile([128, N], F32, tag="f")
            nc.vector.tensor_mul(f[:], d[:], d[:])
            if variant == "full":
                nc.gpsimd.collective_compute(
                    "ReduceScatter",
                    mybir.AluOpType.max,
                    replica_groups=RG,
                    ins=[rs1_in[:].opt()],
                    outs=[rs1_out[:].opt()],
                )
            else:
                nc.sync.dma_start(rs1_out[:], rs1_in[:, 0:RB])
            crow = ep.tile([1, RB], F32, tag="crow")
            nc.sync.dma_start(crow[:], rs1_out[:])
            one1 = ep.tile([1, 1], F32, tag="one1")
            nc.vector.memset(one1[:], 1.0)
            # crow.T via k=1 matmul: out[128,1] = crow[1,128].T @ ones[1,1]
            c_ps = pgp.tile([128, 1], F32, tag="pg", name="c_ps")
            nc.tensor.matmul(c_ps[:], crow[:], one1[:], start=True, stop=True)
            c_own = ep.tile([128, 1], F32, tag="c_own")
            nc.vector.tensor_copy(c_own[:], c_ps[:])
            c2 = ep.tile([128, 1], F32, tag="c2")
            nc.vector.tensor_mul(c2[:], c_own[:], c_own[:])
            # f[p, j] = d^2 - c_own^2 ; M[j] = global max of f over rows
            nc.vector.tensor_scalar(
                f[:], f[:], c2[:], None, op0=mybir.AluOpType.subtract
            )
            pm1 = col_max(f, "pm")
            nc.sync.dma_start(rs2_in[:], pm1[:])
            if variant == "full":
                nc.gpsimd.collective_compute(
                    "ReduceScatter",
                    mybir.AluOpType.max,
                    replica_groups=RG,
                    ins=[rs2_in[:].opt()],
                    outs=[rs2_out[:].opt()],
                )
            else:
                nc.sync.dma_start(rs2_out[:], rs2_in[:, 0:RB])
            m_red = ep.tile([1, RB], F32, tag="m_red")
            nc.sync.dma_start(m_red[:], rs2_out[:])
            # out = scores * exp(-sigma * M) for this core's 128 candidates
            coeff = ep.tile([1, RB], F32, tag="coeff")
            nc.scalar.activation(
                coeff[:], m_red[:], mybir.ActivationFunctionType.Exp, scale=-SIGMA
            )
            outsb = ep.tile([1, RB], F32, tag="outsb")
            nc.vector.tensor_mul(outsb[:], coeff[:], scores[:])
            nc.scalar.dma_start(out_h[:], outsb[:])

    nc.compile()
    return nc


_NC_CACHE = {}


def _get_nc(variant="full"):
    if variant not in _NC_CACHE:
        _NC_CACHE[variant] = build_nc(variant)
    return _NC_CACHE[variant]


def make_in_maps(seg_masks, cate_labels, cate_scores):
    import ml_dtypes

    flat = np.asarray(seg_masks, dtype=np.float32).reshape(N, -1)
    labels = np.asarray(cate_labels)
    scores = np.asarray(cate_scores, dtype=np.float32)
    areas = flat.sum(axis=1)  # exact integers in f32
    xTfull = np.ascontiguousarray(flat.T)  # (40960, 1024)
    gidx = np.arange(N)
    ident = np.eye(128, dtype=np.float32)
    in_maps = []
    for c in range(W):
        rows = slice(c * RB, (c + 1) * RB)
        gr = gidx[rows]
        maskR = (
            (gidx[None, :] > gr[:, None]) & (labels[None, :] == labels[rows][:, None])
        ).astype(np.float32)
        sjsc = areas[None, :] + areas[rows][:, None]  # s_j + s_i, (128, N)
        in_maps.append(
            {
                # partition-major: row p holds k-rows {p, 128+p, ...} of this
                # core's slice; host casts to fp8 (exact for 0/1 masks)
                "xT": np.ascontiguousarray(
                    xTfull[c * KC : (c + 1) * KC]
                    .reshape(KT, 128, N)
                    .transpose(1, 0, 2)
                ).reshape(128, KT * N).astype(ml_dtypes.float8_e4m3fn),
                "maskR": maskR,
                "sjsc": np.ascontiguousarray(sjsc, dtype=np.float32),
                "scores": np.ascontiguousarray(scores[rows].reshape(1, RB)),
                "ident": ident,
            }
        )
    return in_maps


def run_device(in_maps, trace=False):
    nc = _get_nc()
    res = bass_utils.run_bass_kernel_spmd(
        nc, in_maps, core_ids=list(range(W)), trace=trace
    )
    return res


def kernel(seg_masks, cate_labels, cate_scores):
    in_maps = make_in_maps(seg_masks, cate_labels, cate_scores)
    res = run_device(in_maps)
    outs = [np.asarray(res.results[c]["out"]).reshape(RB) for c in range(W)]
    return np.concatenate(outs).astype(np.float32)


# revision 25
# speedup vs baseline: 2.9284x; 1.0026x over previous
"""Matrix NMS (SOLOv2 gaussian decay) on 8 TRN2 NeuronCores.

Strategy: shard the pixel (h*w=40960) contraction dim across the 8 cores.
The host pre-casts the binary masks to fp8 (exact for 0/1), so each core
DMAs only 5.24MB (vs 21MB f32) and the PE consumes it directly with
DoubleRow fp8 matmuls.  Each core computes the partial Gram upper block
rows; block-row a drains as int16 straight into AllToAll shard a (one
contiguous [128, N-128a] write, ~2KB descriptors, copies alternating
DVE/ACT, queues alternating sync/scalar/gpsimd).  After the AllToAll each
core pulls the 8 partials of its own block-row (2KB-descriptor loads on 3
queues) and tree-sums them.  The epilogue is row-oriented: union comes
from a host-precomputed area table (areas are just mask sums — host
input), masked IoU d is formed with one approx-reciprocal pass, and the
two cross-candidate reductions (compensate_iou column max, final decay
max) are done as PE-chunk-transposes + DVE free-dim max, combined across
cores by two 4KB ReduceScatter(max) collectives.  Every core ends up
with the decayed scores for its own 128 candidates; the host concatenates
the 8 slices.
"""

import sys

import numpy as np

for _p in ("/opt/trn_rl_repo",):
    if _p not in sys.path:
        sys.path.insert(0, _p)

from concourse import bacc, bass, mybir, tile
from concourse import bass_utils

N = 1024           # candidates
HWPIX = 160 * 256  # 40960 pixels
W = 8              # cores
KC = HWPIX // W    # 5120 pixel-slice per core
KT = KC // 128     # 40 k-tiles of 128
GRP = 4            # k-tiles per resident SBUF group
RB = 128           # block-row height == shard rows
NP = KT // 2       # 20 k-tile pairs (DoubleRow)
SIGMA = 2.0

F32 = mybir.dt.float32
FP8 = mybir.dt.float8e4  # e4m3: exact for 0/1 mask values
I16 = mybir.dt.int16


def build_nc(variant="full"):
    # variant: "full" = real kernel; "nocc" = collectives replaced by local
    # DMA copies (wrong math, identical local compute/DMA — timing/sim only)
    nc = bacc.Bacc(
        "TRN2", target_bir_lowering=False, debug=False,
        num_devices=W if variant == "full" else 1,
    )

    xT = nc.dram_tensor("xT", [128, KT * N], FP8, kind="ExternalInput")
    maskR_h = nc.dram_tensor("maskR", [RB, N], F32, kind="ExternalInput")
    sjsc_h = nc.dram_tensor("sjsc", [RB, N], F32, kind="ExternalInput")
    scores_h = nc.dram_tensor("scores", [1, RB], F32, kind="ExternalInput")
    ident_h = nc.dram_tensor("ident", [128, 128], F32, kind="ExternalInput")
    out_h = nc.dram_tensor("out", [1, RB], F32, kind="ExternalOutput")

    RG = [list(range(W))]

    with tile.TileContext(nc) as tc:
        with (
            tc.tile_pool(name="dram", bufs=1, space="DRAM") as dramp,
            tc.tile_pool(name="xp", bufs=1) as xp,
            tc.tile_pool(name="pg", bufs=4, space="PSUM") as pgp,
            tc.tile_pool(name="gb", bufs=4) as gbp,
            tc.tile_pool(name="a2al", bufs=1) as alp,
            tc.tile_pool(name="sc", bufs=1) as scp,
            tc.tile_pool(name="epi", bufs=1) as ep,
        ):
            # AllToAll buffers split into column halves: the left half only
            # receives writes from block-rows 0-3, so it ships while blocks
            # 4-7 are still computing/draining.
            HN = N // 2
            cc_h = [dramp.tile([W * RB, HN], I16, tag=f"cc{x}", name=f"cc{x}")
                    for x in range(2)]
            a2a_h = [dramp.tile([W * RB, HN], I16, tag=f"a2a{x}", name=f"a2a{x}")
                     for x in range(2)]
            rs1_in = dramp.tile([1, N], F32, tag="rs1_in")
            rs1_out = dramp.tile([1, RB], F32, tag="rs1_out")
            rs2_in = dramp.tile([1, N], F32, tag="rs2_in")
            rs2_out = dramp.tile([1, RB], F32, tag="rs2_out")

            # constants + epilogue inputs (off the sync queue used by x loads)
            ident = scp.tile([128, 128], F32, tag="ident")
            nc.gpsimd.dma_start(ident[:], ident_h[:])
            scores = scp.tile([1, RB], F32, tag="scores")
            nc.gpsimd.dma_start(scores[:], scores_h[:])
            maskR = scp.tile([128, N], F32, tag="maskR")
            nc.scalar.dma_start(maskR[:], maskR_h[:])
            sjsc = scp.tile([128, N], F32, tag="sjsc")
            nc.scalar.dma_start(sjsc[:], sjsc_h[:])

            # ---- phase 1: fp8 x slice straight into SBUF (10 group loads)
            xg = [xp.tile([128, GRP, N], FP8, tag=f"x{g}", name=f"xg{g}")
                  for g in range(KT // GRP)]
            for g in range(KT // GRP):
                nc.sync.dma_start(xg[g][:], xT[:, g * GRP * N : (g + 1) * GRP * N])

            def xpair(q, c0, c1):
                t = 2 * q
                g, j = t // GRP, t % GRP
                return xg[g][:, j : j + 2, c0:c1]

            def gram_pair(pg, a, q):
                wdt = N - a * 128
                lhsT = xpair(q, a * 128, (a + 1) * 128)
                for off in range(0, wdt, 512):
                    cw = min(512, wdt - off)
                    nc.tensor.matmul(
                        pg[:, off : off + cw],
                        lhsT,
                        xpair(q, a * 128 + off, a * 128 + off + cw),
                        start=(q == 0),
                        stop=(q == NP - 1),
                        perf_mode=mybir.MatmulPerfMode.DoubleRow,
                    )

            def drain(a, pg):
                """PSUM block-row a -> int16 rows of AllToAll shard a.

                Shard a IS block-row a, so the write is one [128, N-128a]
                slab with per-partition-contiguous DRAM runs (up to 2KB).
                The left columns (<128a) of the shard stay garbage; the
                receiver's row mask zeroes them before any reduction.
                """
                wdt = N - a * 128
                gb16 = gbp.tile([128, wdt], I16, tag="gb16")
                if a % 2 == 0:
                    nc.vector.tensor_copy(gb16[:], pg[:, :wdt])
                else:
                    nc.scalar.activation(
                        gb16[:], pg[:, :wdt], mybir.ActivationFunctionType.Copy
                    )
                eng = (nc.sync, nc.scalar, nc.gpsimd)[a % 3]
                rows = slice(a * RB, (a + 1) * RB)
                if a * 128 < HN:
                    lw = HN - a * 128
                    eng.dma_start(cc_h[0][rows, a * 128 : HN], gb16[:, 0:lw])
                    eng.dma_start(cc_h[1][rows, :], gb16[:, lw:wdt])
                else:
                    eng.dma_start(cc_h[1][rows, a * 128 - HN : HN], gb16[:])

            # ---- AllToAll of the left column half (ready after wave A
            # drains); the right half ships after all drains, overlapping
            # the left half's receive-side loads and tree-sum.
            def a2a_chunk(x):
                if variant == "full":
                    nc.gpsimd.collective_compute(
                        "AllToAll",
                        mybir.AluOpType.bypass,
                        replica_groups=RG,
                        ins=[cc_h[x][:].opt()],
                        outs=[a2a_h[x][:].opt()],
                    )
                else:
                    nc.sync.dma_start(a2a_h[x][:], cc_h[x][:])

            # ---- local tree-sum of the 8 partials of this core's block-row
            # (per half: 4 pair-loads on alternating queues, 1KB descriptors)
            summ = ep.tile([128, N], F32, tag="summ")

            def tree_half(x):
                lv = []
                for h in range(4):
                    lt = alp.tile([RB, 2, HN], I16, tag=f"ld{x}{h}", name=f"ld{x}{h}")
                    src = a2a_h[x][2 * h * RB : (2 * h + 2) * RB, :].rearrange(
                        "(s p) n -> p s n", p=RB
                    )
                    eng = (nc.sync, nc.scalar, nc.gpsimd, nc.sync)[h]
                    eng.dma_start(lt[:], src)
                    lv.append(lt)
                m01 = ep.tile([RB, 2, HN], I16, tag=f"m01{x}")
                nc.vector.tensor_add(m01[:], lv[0][:], lv[1][:])
                m23 = ep.tile([RB, 2, HN], I16, tag=f"m23{x}")
                nc.vector.tensor_add(m23[:], lv[2][:], lv[3][:])
                p2 = ep.tile([RB, 2, HN], I16, tag=f"p2{x}")
                nc.vector.tensor_add(p2[:], m01[:], m23[:])
                nc.vector.tensor_add(
                    summ[:, x * HN : (x + 1) * HN], p2[:, 0, :], p2[:, 1, :]
                )

            # ---- phase 2: Gram upper block-rows in two PSUM waves
            wave_a = [pgp.tile([128, N - a * 128], F32, tag="pg", name=f"pgA{a}")
                      for a in range(4)]
            for q in range(NP):
                for a in range(4):
                    gram_pair(wave_a[a], a, q)
            for a in range(4):
                drain(a, wave_a[a])
            a2a_chunk(0)
            for a in range(4, W):
                pg = pgp.tile([128, N - a * 128], F32, tag="pg")
                for q in range(NP):
                    gram_pair(pg, a, q)
                drain(a, pg)
            a2a_chunk(1)
            tree_half(0)
            tree_half(1)

            # ---- row-oriented epilogue on the block-row
            # union = s_i + s_j - inter (host-precomputed s_i+s_j), clamp >= 1
            un = ep.tile([128, N], F32, tag="un")
            nc.vector.tensor_tensor(un[:], sjsc[:], summ[:], op=mybir.AluOpType.subtract)
            nc.vector.tensor_scalar(
                un[:], un[:], 1.0, None, op0=mybir.AluOpType.max
            )
            rec = ep.tile([128, N], F32, tag="rec")
            nc.vector.reciprocal_approx_fast(rec[:], un[:])
            # d[p, j] = masked IoU of own row p vs candidate j
            d = ep.tile([128, N], F32, tag="d")
            nc.vector.tensor_mul(d[:], summ[:], rec[:])
            nc.vector.tensor_mul(d[:], d[:], maskR[:])

            def col_max(src, name):
                """[128, N] -> [W, 128] column max (flattens to global col
                order) via PE chunk transposes + DVE free-dim reduce."""
                tp = pgp.tile([128, W, 128], F32, tag="pg", name=f"{name}_tp")
                for k in range(W):
                    nc.tensor.transpose(
                        tp[:, k, :], src[:, k * 128 : (k + 1) * 128], ident[:]
                    )
                mx8 = ep.tile([128, W], F32, tag=f"{name}_mx8")
                nc.vector.tensor_reduce(
                    mx8[:], tp[:], axis=mybir.AxisListType.X, op=mybir.AluOpType.max
                )
                m8ps = pgp.tile([W, 128], F32, tag="pg", name=f"{name}_m8ps")
                nc.tensor.transpose(m8ps[:], mx8[:], ident[:])
                m8s = ep.tile([W, 128], F32, tag=f"{name}_m8s")
                nc.vector.tensor_copy(m8s[:], m8ps[:])
                return m8s

            # partial compensate_iou (column max over own rows) ->
            # ReduceScatter(max): each core receives compensate for ITS rows
            pc1 = col_max(d, "pc")
            nc.sync.dma_start(rs1_in[:], pc1[:])
            # d^2 on DVE now, so it overlaps the collective below
            f = ep.tile([128, N], F32, tag="f")
            nc.vector.tensor_mul(f[:], d[:], d[:])
            if variant == "full":
                nc.gpsimd.collective_compute(
                    "ReduceScatter",
                    mybir.AluOpType.max,
                    replica_groups=RG,
                    ins=[rs1_in[:].opt()],
                    outs=[rs1_out[:].opt()],
                )
            else:
                nc.sync.dma_start(rs1_out[:], rs1_in[:, 0:RB])
            crow = ep.tile([1, RB], F32, tag="crow")
            nc.sync.dma_start(crow[:], rs1_out[:])
            one1 = ep.tile([1, 1], F32, tag="one1")
            nc.vector.memset(one1[:], 1.0)
            # crow.T via k=1 matmul: out[128,1] = crow[1,128].T @ ones[1,1]
            c_ps = pgp.tile([128, 1], F32, tag="pg", name="c_ps")
            nc.tensor.matmul(c_ps[:], crow[:], one1[:], start=True, stop=True)
            c_own = ep.tile([128, 1], F32, tag="c_own")
            nc.vector.tensor_copy(c_own[:], c_ps[:])
            c2 = ep.tile([128, 1], F32, tag="c2")
            nc.vector.tensor_mul(c2[:], c_own[:], c_own[:])
            # f[p, j] = d^2 - c_own^2 ; M[j] = global max of f over rows
            nc.vector.tensor_scalar(
                f[:], f[:], c2[:], None, op0=mybir.AluOpType.subtract
            )
            pm1 = col_max(f, "pm")
            nc.sync.dma_start(rs2_in[:], pm1[:])
            if variant == "full":
                nc.gpsimd.collective_compute(
                    "ReduceScatter",
                    mybir.AluOpType.max,
                    replica_groups=RG,
                    ins=[rs2_in[:].opt()],
                    outs=[rs2_out[:].opt()],
                )
            else:
                nc.sync.dma_start(rs2_out[:], rs2_in[:, 0:RB])
            m_red = ep.tile([1, RB], F32, tag="m_red")
            nc.sync.dma_start(m_red[:], rs2_out[:])
            # out = scores * exp(-sigma * M) for this core's 128 candidates
            coeff = ep.tile([1, RB], F32, tag="coeff")
            nc.scalar.activation(
                coeff[:], m_red[:], mybir.ActivationFunctionType.Exp, scale=-SIGMA
            )
            outsb = ep.tile([1, RB], F32, tag="outsb")
            nc.vector.tensor_mul(outsb[:], coeff[:], scores[:])
            nc.scalar.dma_start(out_h[:], outsb[:])

    nc.compile()
    return nc


_NC_CACHE = {}


def _get_nc(variant="full"):
    if variant not in _NC_CACHE:
        _NC_CACHE[variant] = build_nc(variant)
    return _NC_CACHE[variant]


def make_in_maps(seg_masks, cate_labels, cate_scores):
    import ml_dtypes

    flat = np.asarray(seg_masks, dtype=np.float32).reshape(N, -1)
    labels = np.asarray(cate_labels)
    scores = np.asarray(cate_scores, dtype=np.float32)
    areas = flat.sum(axis=1)  # exact integers in f32
    xTfull = np.ascontiguousarray(flat.T)  # (40960, 1024)
    gidx = np.arange(N)
    ident = np.eye(128, dtype=np.float32)
    in_maps = []
    for c in range(W):
        rows = slice(c * RB, (c + 1) * RB)
        gr = gidx[rows]
        maskR = (
            (gidx[None, :] > gr[:, None]) & (labels[None, :] == labels[rows][:, None])
        ).astype(np.float32)
        sjsc = areas[None, :] + areas[rows][:, None]  # s_j + s_i, (128, N)
        in_maps.append(
            {
                # partition-major: row p holds k-rows {p, 128+p, ...} of this
                # core's slice; host casts to fp8 (exact for 0/1 masks)
                "xT": np.ascontiguousarray(
                    xTfull[c * KC : (c + 1) * KC]
                    .reshape(KT, 128, N)
                    .transpose(1, 0, 2)
                ).reshape(128, KT * N).astype(ml_dtypes.float8_e4m3fn),
                "maskR": maskR,
                "sjsc": np.ascontiguousarray(sjsc, dtype=np.float32),
                "scores": np.ascontiguousarray(scores[rows].reshape(1, RB)),
                "ident": ident,
            }
        )
    return in_maps


def run_device(in_maps, trace=False):
    nc = _get_nc()
    res = bass_utils.run_bass_kernel_spmd(
        nc, in_maps, core_ids=list(range(W)), trace=trace
    )
    return res


def kernel(seg_masks, cate_labels, cate_scores):
    in_maps = make_in_maps(seg_masks, cate_labels, cate_scores)
    res = run_device(in_maps)
    outs = [np.asarray(res.results[c]["out"]).reshape(RB) for c in range(W)]
    return np.concatenate(outs).astype(np.float32)
